# revision 23
# baseline (speedup 1.0000x reference)
"""Trainium2 Bass kernel for BaseAttention (B=4, S=2048, H=16 heads x 64).

Sharding: 8 cores = 4 batches x 2 head-groups (8 heads / 512 dims each).
Each core computes q/k/v projections for its head group on its batch,
flash-style causal attention (scores never leave the chip), and a partial
o-projection over its 512 head dims. The host sums the two partial outputs
per batch.

Precision strategy (positional hybrid): softmax noise averages out over the
kv accumulation, so rows with many keys tolerate fp8 while early rows
(q < 512, few keys) do not. Query chunk 0 runs fully in float32r. Chunks
1-3 use fp8 MatmulPerfMode.DoubleRow (two 128-row contraction chunks per
instruction at 0.5 cycles/row = 4x the f32r rate) for the q/k/v projections
(e4m3 x and weights) and for P@V (e5m2 probabilities -- raw scores reach
~8.7 sigma on the q.k diagonal so exp() needs e5m2's range -- against e4m3
V). Scores (q@k^T) stay float32r everywhere; the o-projection runs bf16
(same PE rate as f32r, half the SBUF). Softmax denominators come from a
ones column appended to V; normalization multiplies by a PE-broadcast
reciprocal row directly into the SBUF-resident attn^T buffer.
"""

import numpy as np

B = 4
S = 2048
HIDDEN = 1024
NH = 16
DH = 64
HG = 2                  # head groups (cores per batch)
DG = HIDDEN // HG       # 512 dims per group (8 heads)
NCORES = B * HG
SCALE = DH ** -0.5

P = 128
KC = HIDDEN // P        # 8 contraction chunks for projections
NQ = S // 512           # 4 query chunks of 512
SM = S // P             # 16 seq chunks of 128
MCH = DG // P           # 4 chunks of 128 over the group's 512 dims
NHG = NH // HG          # 8 heads per core
NJ = NHG // 2           # 4 head pairs per core

_CACHE = {}


def _emit(nc, tc, tens):
    import concourse.mybir as mybir
    import concourse.bass as bass
    from collections import deque
    from contextlib import ExitStack

    f32 = mybir.dt.float32
    f32r = mybir.dt.float32r
    f8 = mybir.dt.float8e4
    f8e5 = mybir.dt.float8e5
    bf16 = mybir.dt.bfloat16
    DR = mybir.MatmulPerfMode.DoubleRow
    Exp = mybir.ActivationFunctionType.Exp
    mult = mybir.AluOpType.mult
    ds = bass.ds

    xT = tens["xT"].ap().rearrange("(kc p) s -> p kc s", p=P)
    xT32 = tens["xT32"].ap().rearrange("(kc p) s -> p kc s", p=P)
    wqT = tens["wqT"].ap().rearrange("(kc p) d -> p kc d", p=P)
    wkT = tens["wkT"].ap().rearrange("(kc p) d -> p kc d", p=P)
    wvT = tens["wvT"].ap().rearrange("(kc p) d -> p kc d", p=P)
    wqT32 = tens["wqT32"].ap().rearrange("(kc p) d -> p kc d", p=P)
    wkT32 = tens["wkT32"].ap().rearrange("(kc p) d -> p kc d", p=P)
    wvT32 = tens["wvT32"].ap().rearrange("(kc p) d -> p kc d", p=P)
    woT = tens["woT"].ap().rearrange("(ic p) j -> p ic j", p=P)
    masks = tens["masks"].ap().rearrange("t p q -> p t q")
    out = tens["out"].ap().rearrange("(sm p) j -> p sm j", p=P)

    with ExitStack() as ctx:
        persist = ctx.enter_context(tc.tile_pool(name="persist", bufs=1))
        ps_sc = ctx.enter_context(tc.tile_pool(name="ps_sc", bufs=2, space="PSUM"))
        ps_pj = ctx.enter_context(tc.tile_pool(name="ps_pj", bufs=2, space="PSUM"))
        ps_at = ctx.enter_context(tc.tile_pool(name="ps_at", bufs=2, space="PSUM"))
        pstage = ctx.enter_context(tc.tile_pool(name="pstage", bufs=2))
        x32p = ctx.enter_context(tc.tile_pool(name="x32p", bufs=1))
        w32p = ctx.enter_context(tc.tile_pool(name="w32", bufs=2))
        pt32p = ctx.enter_context(tc.tile_pool(name="pt32", bufs=4))
        pt8p = ctx.enter_context(tc.tile_pool(name="pt8", bufs=8))
        recp = ctx.enter_context(tc.tile_pool(name="rec", bufs=1))
        ostp = ctx.enter_context(tc.tile_pool(name="ost", bufs=2))
        qpool = ctx.enter_context(tc.tile_pool(name="qp", bufs=2))

        kT_sb = persist.tile([P, MCH, S], f32r)          # k^T (d on partitions)
        # v + ones column + zero pad (DoubleRow needs pair step % 16 == 0)
        v_sb = persist.tile([P, SM, NHG, DH + 2], f8)
        v32_sb = persist.tile([P, 4, NHG, DH + 1], f32r)  # f32r v for chunk 0
        wq_sb = persist.tile([P, KC, DG], f8)
        wk_sb = persist.tile([P, KC, DG], f8)
        wv_sb = persist.tile([P, KC, DG], f8)
        wo_sb = persist.tile([P, MCH, HIDDEN], bf16)
        attnT_sb = persist.tile([P, MCH, S], bf16)       # attn^T, SBUF-resident
        mask_sb = persist.tile([P, 2, 1024], bf16)

        ones_f32 = persist.tile([P, 1], f32)
        zeros_f32 = persist.tile([P, 1], f32)
        nc.vector.memset(ones_f32[:], 1.0)  # f32r memset fails ISA checks
        nc.vector.memset(zeros_f32[:], 0.0)
        with nc.allow_low_precision(reason="ones column"):
            nc.vector.tensor_copy(
                out=v_sb[:, :, :, DH:DH + 1],
                in_=ones_f32[:, 0:1].to_broadcast([P, SM, NHG, 1]),
            )
            nc.vector.tensor_copy(
                out=v_sb[:, :, :, DH + 1:DH + 2],
                in_=zeros_f32[:, 0:1].to_broadcast([P, SM, NHG, 1]),
            )
            nc.vector.tensor_copy(
                out=v32_sb[:, :, :, DH:DH + 1],
                in_=ones_f32[:, 0:1].to_broadcast([P, 4, NHG, 1]),
            )

        xts = {}
        qsbs = {}

        def proj_closures(n):
            """q/k/v projection work for seq chunk n: 13 closures.

            Chunk 0 runs f32r from transient f32r x/w staging; later chunks
            run fp8 DoubleRow from the persistent e4m3 x/w tiles.
            """
            cls = []
            w32 = {}

            if n == 0:
                def load_xt0():
                    xt = x32p.tile([P, KC, 512], f32r, tag="xt32")
                    for hh in range(4):
                        hsl = ds(hh * (KC // 4), KC // 4)
                        nc.sync.dma_start(xt[:, hsl, :], xT32[:, hsl, :])
                    xts[0] = xt
                    qsbs[0] = qpool.tile([P, MCH, 512], f32r, tag="qsb",
                                         name="qsb_0")
                    w32["q"] = w32p.tile([P, KC, DG], f32r, tag="w32",
                                         name="w32_q")
                    for c4 in range(4):
                        csl = ds(c4 * (DG // 4), DG // 4)
                        nc.sync.dma_start(w32["q"][:, :, csl], wqT32[:, :, csl])
                cls.append(load_xt0)
            else:
                def load_xt(n=n):
                    xt = pstage.tile([P, KC, 512], f8, tag="xt")
                    for hh in range(4):
                        hsl = ds(hh * (KC // 4), KC // 4)
                        nc.sync.dma_start(xt[:, hsl, :], xT[:, hsl, ds(n * 512, 512)])
                    xts[n] = xt
                    qsbs[n] = qpool.tile([P, MCH, 512], f32r, tag="qsb",
                                         name=f"qsb_{n}")
                cls.append(load_xt)

            def qk_mm(n, w_sb, m, xt, ps):
                if n == 0:
                    for kc in range(KC):
                        nc.tensor.matmul(
                            ps[:], w_sb[:, kc, ds(m * P, P)],
                            xt[:, kc, :],
                            start=(kc == 0), stop=(kc == KC - 1),
                        )
                else:
                    for i in range(KC // 2):
                        nc.tensor.matmul(
                            ps[:], w_sb[:, ds(2 * i, 2), ds(m * P, P)],
                            xt[:, ds(2 * i, 2), :],
                            start=(i == 0), stop=(i == KC // 2 - 1),
                            perf_mode=DR,
                        )

            for wname, dst_name in (("q", "q"), ("k", "k")):
                for m in range(MCH):
                    def qk_group(n=n, wname=wname, dst_name=dst_name, m=m):
                        xt = xts[n]
                        if n == 0:
                            w_sb = w32[wname]
                        else:
                            w_sb = wq_sb if wname == "q" else wk_sb
                        ps = ps_pj.tile([P, 512], f32, tag="pj")
                        qk_mm(n, w_sb, m, xt, ps)
                        if dst_name == "k":
                            nc.vector.tensor_copy(
                                out=kT_sb[:, m, ds(n * 512, 512)], in_=ps[:]
                            )
                        else:
                            nc.vector.tensor_copy(
                                out=qsbs[n][:, m, :], in_=ps[:]
                            )
                        # stage the next transient f32r weight while chunk-0
                        # groups still run (separate w32 buffer)
                        if n == 0 and wname == "q" and m == 0:
                            w32["k"] = w32p.tile([P, KC, DG], f32r, tag="w32",
                                                 name="w32_k")
                            for c4 in range(4):
                                csl = ds(c4 * (DG // 4), DG // 4)
                                nc.sync.dma_start(w32["k"][:, :, csl],
                                                  wkT32[:, :, csl])
                        if n == 0 and wname == "k" and m == 0:
                            w32["v"] = w32p.tile([P, KC, DG], f32r, tag="w32",
                                                 name="w32_v")
                            for c4 in range(4):
                                csl = ds(c4 * (DG // 4), DG // 4)
                                nc.sync.dma_start(w32["v"][:, :, csl],
                                                  wvT32[:, :, csl])
                    cls.append(qk_group)

            for sm in range(4 * n, 4 * n + 4):
                def v_group(n=n, sm=sm):
                    xt = xts[n]
                    ps = ps_pj.tile([P, 512], f32, tag="pj")
                    if n == 0:
                        for kc in range(KC):
                            nc.tensor.matmul(
                                ps[:], xt[:, kc, ds(sm * P, P)],
                                w32["v"][:, kc, :],
                                start=(kc == 0), stop=(kc == KC - 1),
                            )
                    else:
                        for i in range(KC // 2):
                            nc.tensor.matmul(
                                ps[:],
                                xt[:, ds(2 * i, 2), ds((sm - 4 * n) * P, P)],
                                wv_sb[:, ds(2 * i, 2), :],
                                start=(i == 0), stop=(i == KC // 2 - 1),
                                perf_mode=DR,
                            )
                    with nc.allow_low_precision(reason="v fp8 store"):
                        nc.vector.tensor_copy(
                            out=v_sb[:, sm, :, 0:DH],
                            in_=ps[:].rearrange("p (h d) -> p h d", h=NHG),
                        )
                        if n == 0:
                            nc.vector.tensor_copy(
                                out=v32_sb[:, sm, :, 0:DH],
                                in_=ps[:].rearrange("p (h d) -> p h d",
                                                    h=NHG),
                            )
                cls.append(v_group)
            return cls

        def oproj_closures(n):
            """o-projection (bf16) for seq chunk n (attn^T already in SBUF)."""
            cls = []
            for sm in range(4 * n, 4 * n + 4):
                for j2 in range(2):
                    def o_group(sm=sm, j2=j2):
                        ps = ps_pj.tile([P, 512], f32, tag="pj")
                        for ic in range(MCH):
                            nc.tensor.matmul(
                                ps[:],
                                attnT_sb[:, ic, ds(sm * P, P)],
                                wo_sb[:, ic, ds(j2 * 512, 512)],
                                start=(ic == 0), stop=(ic == MCH - 1),
                            )
                        ost = ostp.tile([P, 512], f32, tag="ost")
                        nc.vector.tensor_copy(out=ost[:], in_=ps[:])
                        nc.sync.dma_start(out[:, sm, ds(j2 * 512, 512)], ost[:])
                    cls.append(o_group)
            return cls

        # startup: x(0)+wq f32r first so the PE starts quickly; wk/wv f32r
        # stream during the chunk-0 groups; fp8 weights + wo follow.
        p0 = proj_closures(0)
        p0[0]()                                   # xt32 + wq32 DMA
        for c in p0[1:5]:                         # q groups (starts wk32 DMA)
            c()
        for c in p0[5:9]:                         # k groups (starts wv32 DMA)
            c()
        nc.sync.dma_start(mask_sb[:], masks)
        for c in p0[9:]:                          # v groups
            c()
        for w_sb, wT in ((wq_sb, wqT), (wk_sb, wkT), (wv_sb, wvT)):
            for c4 in range(2):
                csl = ds(c4 * (DG // 2), DG // 2)
                nc.sync.dma_start(w_sb[:, :, csl], wT[:, :, csl])
        for c4 in range(4):
            csl = ds(c4 * (HIDDEN // 4), HIDDEN // 4)
            nc.sync.dma_start(wo_sb[:, :, csl], woT[:, :, csl])

        filler = deque()
        pending = []  # deferred normalization closures

        def flush_pending():
            for c in pending:
                c()
            pending.clear()

        def norm_closure(n, j, e, acc):
            qsl = ds(n * 512, 512)

            def finish():
                # reciprocal of the denominator row (partition 64), replicate
                # it down 64 partitions on the (otherwise idle) GPSIMD engine,
                # then normalize straight into the SBUF attn^T buffer
                # (cross-partition-base write for e=1).
                # denominator row to partition 0 (partition_broadcast only
                # reads physical partition 0 on HW, and reciprocal_approx
                # can't handle a cross-base PSUM read itself)
                row0 = recp.tile([1, 512], f32, tag="row0")
                nc.vector.tensor_copy(out=row0[:], in_=acc[DH:DH + 1, :])
                rcp32 = recp.tile([1, 512], f32, tag="rcp32")
                nc.vector.reciprocal_approx_fast(rcp32[:], row0[:])
                bc = recp.tile([DH, 512], f32, tag="bc")
                nc.gpsimd.partition_broadcast(bc[:], rcp32[:])
                with nc.allow_low_precision(reason="attn bf16 store"):
                    nc.vector.tensor_tensor(
                        attnT_sb[ds(e * DH, DH), j, qsl],
                        acc[0:DH, :], bc[:], mult)
            return finish

        for n in range(NQ):
            if n + 1 < NQ:
                pc = proj_closures(n + 1)
                pc[0]()                       # start xt(n+1) DMA immediately
                filler.extend(pc[1:])
            if n == 2:
                filler.extend(oproj_closures(0))
            elif n == 3:
                filler.extend(oproj_closures(1))
                filler.extend(oproj_closures(2))
            npairs = 2 * (n + 1)
            total_pairs = NJ * npairs
            pace_num = len(filler)
            pace_acc = 0
            for j in range(NJ):
                acc = [
                    ps_at.tile([DH + 2, 512], f32, tag="acc",
                               name=f"acc_{n}_{j}_{e}")
                    for e in range(2)
                ]
                defer = 2 if n == 0 else 3
                pvq = []  # deferred PV matmuls (consumed `defer` pairs later)

                def emit_pv(n=n, j=j):
                    tp, e, pt = pvq.pop(0)
                    h = 2 * j + e
                    if n == 0:
                        for u in range(2):
                            nc.tensor.matmul(
                                acc[e][0:DH + 1, :],
                                v32_sb[:, 2 * tp + u, h, :],
                                pt[:, ds(u * 512, 512)],
                                start=(tp == 0 and u == 0),
                                stop=(tp == npairs - 1 and u == 1),
                            )
                    else:
                        nc.tensor.matmul(
                            acc[e][:],
                            v_sb[:, ds(2 * tp, 2), h, :],
                            pt[:, :].rearrange("p (i u) -> p i u", i=2),
                            start=(tp == 0),
                            stop=(tp == npairs - 1),
                            perf_mode=DR,
                        )

                for t in range(npairs):
                    if pending:
                        flush_pending()
                    # pump interleaved proj/o-proj work in bursts so the PE
                    # stays dense (clock-gate ramp needs >3us stretches)
                    pace_acc += pace_num
                    while pace_acc >= total_pairs and filler:
                        filler.popleft()()
                        pace_acc -= total_pairs
                    new_pvq = []
                    for e in range(2):          # head pair member
                        bp = e * DH             # base partition 0/64
                        ps = ps_sc.tile([P, 1024], f32, tag="sc")
                        for u in range(2):      # m-pair member
                            m = 2 * t + u
                            nc.tensor.matmul(
                                ps[:, ds(u * 512, 512)],
                                kT_sb[bp:bp + DH, j, ds(m * P, P)],
                                qsbs[n][bp:bp + DH, j, :],
                                start=True, stop=True,
                            )
                            if pvq and pvq[0][0] <= t - defer:
                                emit_pv()
                        if n == 0:
                            pt = pt32p.tile([P, 1024], f32r, tag="pt32")
                        else:
                            pt = pt8p.tile([P, 1024], f8e5, tag="pt8")
                        with nc.allow_low_precision(reason="fp8 probs"):
                            nc.scalar.activation(pt[:], ps[:], Exp, scale=SCALE)
                            if t >= 2 * n:      # diagonal pair: mask
                                nc.vector.tensor_tensor(
                                    pt[:], pt[:], mask_sb[:, t - 2 * n, :], mult
                                )
                        if pvq and pvq[0][0] <= t - defer:
                            emit_pv()
                        new_pvq.append((t, e, pt))
                    while pvq and pvq[0][0] <= t - defer:
                        emit_pv()
                    pvq.extend(new_pvq)
                while pvq:
                    emit_pv()
                for e in range(2):
                    pending.append(norm_closure(n, j, e, acc[e]))
            while filler:
                filler.popleft()()
        flush_pending()
        for c in oproj_closures(NQ - 1):
            c()


def _build():
    import concourse.mybir as mybir
    import concourse.tile as tile
    from concourse import bacc

    f32 = mybir.dt.float32
    f32r = mybir.dt.float32r
    f8 = mybir.dt.float8e4
    nc = bacc.Bacc("TRN2", target_bir_lowering=False, debug=False,
                   num_devices=NCORES)
    tens = {
        "xT": nc.dram_tensor("xT", [HIDDEN, S], f8, kind="ExternalInput"),
        "xT32": nc.dram_tensor("xT32", [HIDDEN, 512], f32r,
                               kind="ExternalInput"),
        "wqT": nc.dram_tensor("wqT", [HIDDEN, DG], f8, kind="ExternalInput"),
        "wkT": nc.dram_tensor("wkT", [HIDDEN, DG], f8, kind="ExternalInput"),
        "wvT": nc.dram_tensor("wvT", [HIDDEN, DG], f8, kind="ExternalInput"),
        "wqT32": nc.dram_tensor("wqT32", [HIDDEN, DG], f32r,
                                kind="ExternalInput"),
        "wkT32": nc.dram_tensor("wkT32", [HIDDEN, DG], f32r,
                                kind="ExternalInput"),
        "wvT32": nc.dram_tensor("wvT32", [HIDDEN, DG], f32r,
                                kind="ExternalInput"),
        "woT": nc.dram_tensor("woT", [DG, HIDDEN], mybir.dt.bfloat16,
                              kind="ExternalInput"),
        "masks": nc.dram_tensor("masks", [2, P, 1024], mybir.dt.bfloat16,
                                kind="ExternalInput"),
        "out": nc.dram_tensor("out", [S, HIDDEN], f32, kind="ExternalOutput"),
    }
    with tile.TileContext(nc) as tc:
        _emit(nc, tc, tens)
    nc.compile()
    return nc


def get_program():
    if "nc" not in _CACHE:
        _CACHE["nc"] = _build()
    return _CACHE["nc"]


def make_in_maps(hidden_states, attention_mask, wq, wk, wv, wo):
    """Build the per-core input maps (host-side sharding)."""
    import ml_dtypes
    f8 = ml_dtypes.float8_e4m3
    bf = ml_dtypes.bfloat16

    hidden_states = np.asarray(hidden_states, dtype=np.float32)
    attention_mask = np.asarray(attention_mask, dtype=np.float32)
    wq = np.asarray(wq, dtype=np.float32)
    wk = np.asarray(wk, dtype=np.float32)
    wv = np.asarray(wv, dtype=np.float32)
    wo = np.asarray(wo, dtype=np.float32)

    # Pair-level mask tiles for the diagonal blocks of scores^T, derived from
    # the provided additive mask (0 = attend, big negative = blocked).
    am = attention_mask[0, 0]
    mask_np = np.empty((2, P, 1024), dtype=np.float32)
    for t in range(2):
        for u in range(2):
            off = (2 * t + u) * P
            blk = (am[512:1024, 512 + off:512 + off + P] == 0.0)
            mask_np[t, :, u * 512:(u + 1) * 512] = blk.T.astype(np.float32)
    mask_np = mask_np.astype(bf)

    in_maps = []
    for c in range(NCORES):
        b, g = divmod(c, HG)
        rows = slice(g * DG, (g + 1) * DG)
        xt = np.ascontiguousarray(hidden_states[b].T)
        in_maps.append({
            "xT": xt.astype(f8),
            "xT32": np.ascontiguousarray(xt[:, :512]),
            "wqT": np.ascontiguousarray(wq[rows, :].T).astype(f8),
            "wkT": np.ascontiguousarray(wk[rows, :].T).astype(f8),
            "wvT": np.ascontiguousarray(wv[rows, :].T).astype(f8),
            "wqT32": np.ascontiguousarray(wq[rows, :].T),
            "wkT32": np.ascontiguousarray(wk[rows, :].T),
            "wvT32": np.ascontiguousarray(wv[rows, :].T),
            "woT": np.ascontiguousarray(wo[:, rows].T).astype(bf),
            "masks": mask_np,
        })
    return in_maps


def combine_outputs(results):
    out = np.empty((B, S, HIDDEN), dtype=np.float32)
    for b in range(B):
        out[b] = results[HG * b]["out"] + results[HG * b + 1]["out"]
    return out


def kernel(hidden_states, attention_mask, wq, wk, wv, wo):
    from concourse.bass_utils import run_bass_kernel_spmd

    nc = get_program()
    in_maps = make_in_maps(hidden_states, attention_mask, wq, wk, wv, wo)
    res = run_bass_kernel_spmd(nc, in_maps, list(range(NCORES)))
    return combine_outputs(res.results)


# revision 25
# speedup vs baseline: 1.0392x; 1.0392x over previous
"""Trainium2 Bass kernel for BaseAttention (B=4, S=2048, H=16 heads x 64).

Sharding: 8 cores = 4 batches x 2 head-groups (8 heads / 512 dims each).
Each core computes q/k/v projections for its head group on its batch,
flash-style causal attention (scores never leave the chip), and a partial
o-projection over its 512 head dims. The host sums the two partial outputs
per batch.

Precision strategy (positional hybrid): softmax noise averages out over the
kv accumulation, so rows with many keys tolerate fp8 while early rows
(q < 512, few keys) do not. Query chunk 0 runs fully in float32r. Chunks
1-3 use fp8 MatmulPerfMode.DoubleRow (two 128-row contraction chunks per
instruction at 0.5 cycles/row = 4x the f32r rate) for the q/k/v projections
(e4m3 x and weights) and for P@V (e5m2 probabilities -- raw scores reach
~8.7 sigma on the q.k diagonal so exp() needs e5m2's range -- against e4m3
V). Scores (q@k^T) stay float32r everywhere; the o-projection runs bf16
(same PE rate as f32r, half the SBUF). Softmax denominators come from a
ones column appended to V; normalization multiplies by a PE-broadcast
reciprocal row directly into the SBUF-resident attn^T buffer.
"""

import numpy as np

B = 4
S = 2048
HIDDEN = 1024
NH = 16
DH = 64
HG = 2                  # head groups (cores per batch)
DG = HIDDEN // HG       # 512 dims per group (8 heads)
NCORES = B * HG
SCALE = DH ** -0.5

P = 128
KC = HIDDEN // P        # 8 contraction chunks for projections
NQ = S // 512           # 4 query chunks of 512
SM = S // P             # 16 seq chunks of 128
MCH = DG // P           # 4 chunks of 128 over the group's 512 dims
NHG = NH // HG          # 8 heads per core
NJ = NHG // 2           # 4 head pairs per core

_CACHE = {}


def _emit(nc, tc, tens):
    import concourse.mybir as mybir
    import concourse.bass as bass
    from collections import deque
    from contextlib import ExitStack

    f32 = mybir.dt.float32
    f32r = mybir.dt.float32r
    f8 = mybir.dt.float8e4
    f8e5 = mybir.dt.float8e5
    bf16 = mybir.dt.bfloat16
    DR = mybir.MatmulPerfMode.DoubleRow
    Exp = mybir.ActivationFunctionType.Exp
    mult = mybir.AluOpType.mult
    ds = bass.ds

    xT = tens["xT"].ap().rearrange("(kc p) s -> p kc s", p=P)
    xT32 = tens["xT32"].ap().rearrange("(kc p) s -> p kc s", p=P)
    wqT = tens["wqT"].ap().rearrange("(kc p) d -> p kc d", p=P)
    wkT = tens["wkT"].ap().rearrange("(kc p) d -> p kc d", p=P)
    wvT = tens["wvT"].ap().rearrange("(kc p) d -> p kc d", p=P)
    wqT32 = tens["wqT32"].ap().rearrange("(kc p) d -> p kc d", p=P)
    wkT32 = tens["wkT32"].ap().rearrange("(kc p) d -> p kc d", p=P)
    wvT32 = tens["wvT32"].ap().rearrange("(kc p) d -> p kc d", p=P)
    woT = tens["woT"].ap().rearrange("(ic p) j -> p ic j", p=P)
    masks = tens["masks"].ap().rearrange("t p q -> p t q")
    out = tens["out"].ap().rearrange("(sm p) j -> p sm j", p=P)

    with ExitStack() as ctx:
        persist = ctx.enter_context(tc.tile_pool(name="persist", bufs=1))
        ps_mm = ctx.enter_context(tc.tile_pool(name="ps_mm", bufs=3, space="PSUM"))
        ps_at = ctx.enter_context(tc.tile_pool(name="ps_at", bufs=2, space="PSUM"))
        pstage = ctx.enter_context(tc.tile_pool(name="pstage", bufs=2))
        x32p = ctx.enter_context(tc.tile_pool(name="x32p", bufs=1))
        w32p = ctx.enter_context(tc.tile_pool(name="w32", bufs=2))
        pt32p = ctx.enter_context(tc.tile_pool(name="pt32", bufs=4))
        pt8p = ctx.enter_context(tc.tile_pool(name="pt8", bufs=8))
        recp = ctx.enter_context(tc.tile_pool(name="rec", bufs=1))
        ostp = ctx.enter_context(tc.tile_pool(name="ost", bufs=2))
        qpool = ctx.enter_context(tc.tile_pool(name="qp", bufs=2))

        kT_sb = persist.tile([P, MCH, S], f32r)          # k^T (d on partitions)
        # v + ones column + zero pad (DoubleRow needs pair step % 16 == 0)
        v_sb = persist.tile([P, SM, NHG, DH + 2], f8)
        v32_sb = persist.tile([P, 4, NHG, DH + 1], f32r)  # f32r v for chunk 0
        wq_sb = persist.tile([P, KC, DG], f8)
        wk_sb = persist.tile([P, KC, DG], f8)
        wv_sb = persist.tile([P, KC, DG], f8)
        wo_sb = persist.tile([P, MCH, HIDDEN], bf16)
        attnT_sb = persist.tile([P, MCH, S], bf16)       # attn^T, SBUF-resident
        mask_sb = persist.tile([P, 2, 1024], bf16)

        ones_f32 = persist.tile([P, 1], f32)
        zeros_f32 = persist.tile([P, 1], f32)
        nc.vector.memset(ones_f32[:], 1.0)  # f32r memset fails ISA checks
        nc.vector.memset(zeros_f32[:], 0.0)
        with nc.allow_low_precision(reason="ones column"):
            nc.vector.tensor_copy(
                out=v_sb[:, :, :, DH:DH + 1],
                in_=ones_f32[:, 0:1].to_broadcast([P, SM, NHG, 1]),
            )
            nc.vector.tensor_copy(
                out=v_sb[:, :, :, DH + 1:DH + 2],
                in_=zeros_f32[:, 0:1].to_broadcast([P, SM, NHG, 1]),
            )
            nc.vector.tensor_copy(
                out=v32_sb[:, :, :, DH:DH + 1],
                in_=ones_f32[:, 0:1].to_broadcast([P, 4, NHG, 1]),
            )

        xts = {}
        qsbs = {}

        def proj_closures(n):
            """q/k/v projection work for seq chunk n: 13 closures.

            Chunk 0 runs f32r from transient f32r x/w staging; later chunks
            run fp8 DoubleRow from the persistent e4m3 x/w tiles.
            """
            cls = []
            w32 = {}

            if n == 0:
                def load_xt0():
                    xt = x32p.tile([P, KC, 512], f32r, tag="xt32")
                    for hh in range(4):
                        hsl = ds(hh * (KC // 4), KC // 4)
                        nc.sync.dma_start(xt[:, hsl, :], xT32[:, hsl, :])
                    xts[0] = xt
                    qsbs[0] = qpool.tile([P, MCH, 512], f32r, tag="qsb",
                                         name="qsb_0")
                    w32["q"] = w32p.tile([P, KC, DG], f32r, tag="w32",
                                         name="w32_q")
                    for c4 in range(4):
                        csl = ds(c4 * (DG // 4), DG // 4)
                        nc.sync.dma_start(w32["q"][:, :, csl], wqT32[:, :, csl])
                cls.append(load_xt0)
            else:
                def load_xt(n=n):
                    xt = pstage.tile([P, KC, 512], f8, tag="xt")
                    for hh in range(4):
                        hsl = ds(hh * (KC // 4), KC // 4)
                        nc.sync.dma_start(xt[:, hsl, :], xT[:, hsl, ds(n * 512, 512)])
                    xts[n] = xt
                    qsbs[n] = qpool.tile([P, MCH, 512], f32r, tag="qsb",
                                         name=f"qsb_{n}")
                cls.append(load_xt)

            def qk_mm(n, w_sb, m, xt, ps):
                if n == 0:
                    for kc in range(KC):
                        nc.tensor.matmul(
                            ps[:, :512], w_sb[:, kc, ds(m * P, P)],
                            xt[:, kc, :],
                            start=(kc == 0), stop=(kc == KC - 1),
                        )
                else:
                    for i in range(KC // 2):
                        nc.tensor.matmul(
                            ps[:, :512], w_sb[:, ds(2 * i, 2), ds(m * P, P)],
                            xt[:, ds(2 * i, 2), :],
                            start=(i == 0), stop=(i == KC // 2 - 1),
                            perf_mode=DR,
                        )

            for wname, dst_name in (("q", "q"), ("k", "k")):
                for m in range(MCH):
                    def qk_group(n=n, wname=wname, dst_name=dst_name, m=m):
                        xt = xts[n]
                        if n == 0:
                            w_sb = w32[wname]
                        else:
                            w_sb = wq_sb if wname == "q" else wk_sb
                        ps = ps_mm.tile([P, 1024], f32, tag="mm")
                        qk_mm(n, w_sb, m, xt, ps)
                        if dst_name == "k":
                            nc.vector.tensor_copy(
                                out=kT_sb[:, m, ds(n * 512, 512)], in_=ps[:, :512]
                            )
                        else:
                            nc.vector.tensor_copy(
                                out=qsbs[n][:, m, :], in_=ps[:, :512]
                            )
                        # stage the next transient f32r weight while chunk-0
                        # groups still run (separate w32 buffer)
                        if n == 0 and wname == "q" and m == 0:
                            w32["k"] = w32p.tile([P, KC, DG], f32r, tag="w32",
                                                 name="w32_k")
                            for c4 in range(4):
                                csl = ds(c4 * (DG // 4), DG // 4)
                                nc.sync.dma_start(w32["k"][:, :, csl],
                                                  wkT32[:, :, csl])
                        if n == 0 and wname == "k" and m == 0:
                            w32["v"] = w32p.tile([P, KC, DG], f32r, tag="w32",
                                                 name="w32_v")
                            for c4 in range(4):
                                csl = ds(c4 * (DG // 4), DG // 4)
                                nc.sync.dma_start(w32["v"][:, :, csl],
                                                  wvT32[:, :, csl])
                    cls.append(qk_group)

            for sm in range(4 * n, 4 * n + 4):
                def v_group(n=n, sm=sm):
                    xt = xts[n]
                    ps = ps_mm.tile([P, 1024], f32, tag="mm")
                    if n == 0:
                        for kc in range(KC):
                            nc.tensor.matmul(
                                ps[:, :512], xt[:, kc, ds(sm * P, P)],
                                w32["v"][:, kc, :],
                                start=(kc == 0), stop=(kc == KC - 1),
                            )
                    else:
                        for i in range(KC // 2):
                            nc.tensor.matmul(
                                ps[:, :512],
                                xt[:, ds(2 * i, 2), ds((sm - 4 * n) * P, P)],
                                wv_sb[:, ds(2 * i, 2), :],
                                start=(i == 0), stop=(i == KC // 2 - 1),
                                perf_mode=DR,
                            )
                    with nc.allow_low_precision(reason="v fp8 store"):
                        nc.vector.tensor_copy(
                            out=v_sb[:, sm, :, 0:DH],
                            in_=ps[:, :512].rearrange("p (h d) -> p h d", h=NHG),
                        )
                        if n == 0:
                            nc.vector.tensor_copy(
                                out=v32_sb[:, sm, :, 0:DH],
                                in_=ps[:, :512].rearrange("p (h d) -> p h d",
                                                          h=NHG),
                            )
                cls.append(v_group)
            return cls

        def oproj_closures(n):
            """o-projection (bf16) for seq chunk n (attn^T already in SBUF)."""
            cls = []
            for sm in range(4 * n, 4 * n + 4):
                for j2 in range(2):
                    def o_group(sm=sm, j2=j2):
                        ps = ps_mm.tile([P, 1024], f32, tag="mm")
                        for ic in range(MCH):
                            nc.tensor.matmul(
                                ps[:, :512],
                                attnT_sb[:, ic, ds(sm * P, P)],
                                wo_sb[:, ic, ds(j2 * 512, 512)],
                                start=(ic == 0), stop=(ic == MCH - 1),
                            )
                        ost = ostp.tile([P, 512], f32, tag="ost")
                        nc.vector.tensor_copy(out=ost[:], in_=ps[:, :512])
                        nc.sync.dma_start(out[:, sm, ds(j2 * 512, 512)], ost[:])
                    cls.append(o_group)
            return cls

        # startup: x(0)+wq f32r first so the PE starts quickly; wk/wv f32r
        # stream during the chunk-0 groups; fp8 weights + wo follow.
        p0 = proj_closures(0)
        p0[0]()                                   # xt32 + wq32 DMA
        for c in p0[1:5]:                         # q groups (starts wk32 DMA)
            c()
        for c in p0[5:9]:                         # k groups (starts wv32 DMA)
            c()
        nc.sync.dma_start(mask_sb[:], masks)
        for c in p0[9:]:                          # v groups
            c()
        for w_sb, wT in ((wq_sb, wqT), (wk_sb, wkT), (wv_sb, wvT)):
            for c4 in range(2):
                csl = ds(c4 * (DG // 2), DG // 2)
                nc.sync.dma_start(w_sb[:, :, csl], wT[:, :, csl])
        for c4 in range(4):
            csl = ds(c4 * (HIDDEN // 4), HIDDEN // 4)
            nc.sync.dma_start(wo_sb[:, :, csl], woT[:, :, csl])

        filler = deque()
        pending = []  # deferred normalization closures

        def flush_pending():
            for c in pending:
                c()
            pending.clear()

        def norm_closure(n, j, e, acc):
            qsl = ds(n * 512, 512)

            def finish():
                # reciprocal of the denominator row (partition 64), replicate
                # it down 64 partitions on the (otherwise idle) GPSIMD engine,
                # then normalize straight into the SBUF attn^T buffer
                # (cross-partition-base write for e=1).
                # denominator row to partition 0 (partition_broadcast only
                # reads physical partition 0 on HW, and reciprocal_approx
                # can't handle a cross-base PSUM read itself)
                row0 = recp.tile([1, 512], f32, tag="row0")
                nc.vector.tensor_copy(out=row0[:], in_=acc[DH:DH + 1, :])
                rcp32 = recp.tile([1, 512], f32, tag="rcp32")
                nc.vector.reciprocal_approx_fast(rcp32[:], row0[:])
                bc = recp.tile([DH, 512], f32, tag="bc")
                nc.gpsimd.partition_broadcast(bc[:], rcp32[:])
                with nc.allow_low_precision(reason="attn bf16 store"):
                    nc.vector.tensor_tensor(
                        attnT_sb[ds(e * DH, DH), j, qsl],
                        acc[0:DH, :], bc[:], mult)
            return finish

        for n in range(NQ):
            if n + 1 < NQ:
                pc = proj_closures(n + 1)
                pc[0]()                       # start xt(n+1) DMA immediately
                filler.extend(pc[1:])
            if n == 2:
                filler.extend(oproj_closures(0))
            elif n == 3:
                filler.extend(oproj_closures(1))
                filler.extend(oproj_closures(2))
            npairs = 2 * (n + 1)
            total_pairs = NJ * npairs
            pace_num = len(filler)
            pace_acc = 0
            for j in range(NJ):
                acc = [
                    ps_at.tile([DH + 2, 512], f32, tag="acc",
                               name=f"acc_{n}_{j}_{e}")
                    for e in range(2)
                ]
                defer = 2 if n == 0 else 3
                pvq = []  # deferred PV matmuls (consumed `defer` pairs later)

                def emit_pv(n=n, j=j):
                    tp, e, pt = pvq.pop(0)
                    h = 2 * j + e
                    if n == 0:
                        for u in range(2):
                            nc.tensor.matmul(
                                acc[e][0:DH + 1, :],
                                v32_sb[:, 2 * tp + u, h, :],
                                pt[:, ds(u * 512, 512)],
                                start=(tp == 0 and u == 0),
                                stop=(tp == npairs - 1 and u == 1),
                            )
                    else:
                        nc.tensor.matmul(
                            acc[e][:],
                            v_sb[:, ds(2 * tp, 2), h, :],
                            pt[:, :].rearrange("p (i u) -> p i u", i=2),
                            start=(tp == 0),
                            stop=(tp == npairs - 1),
                            perf_mode=DR,
                        )

                for t in range(npairs):
                    # pump interleaved proj/o-proj work in bursts so the PE
                    # stays dense (clock-gate ramp needs >3us stretches)
                    pace_acc += pace_num
                    while pace_acc >= total_pairs and filler:
                        filler.popleft()()
                        pace_acc -= total_pairs
                    new_pvq = []
                    for e in range(2):          # head pair member
                        bp = e * DH             # base partition 0/64
                        ps = ps_mm.tile([P, 1024], f32, tag="mm")
                        for u in range(2):      # m-pair member
                            m = 2 * t + u
                            nc.tensor.matmul(
                                ps[:, ds(u * 512, 512)],
                                kT_sb[bp:bp + DH, j, ds(m * P, P)],
                                qsbs[n][bp:bp + DH, j, :],
                                start=True, stop=True,
                            )
                            if pvq and pvq[0][0] <= t - defer:
                                emit_pv()
                        if n == 0:
                            pt = pt32p.tile([P, 1024], f32r, tag="pt32")
                        else:
                            pt = pt8p.tile([P, 1024], f8e5, tag="pt8")
                        with nc.allow_low_precision(reason="fp8 probs"):
                            nc.scalar.activation(pt[:], ps[:], Exp, scale=SCALE)
                            if t >= 2 * n:      # diagonal pair: mask
                                nc.vector.tensor_tensor(
                                    pt[:], pt[:], mask_sb[:, t - 2 * n, :], mult
                                )
                        if pvq and pvq[0][0] <= t - defer:
                            emit_pv()
                        new_pvq.append((t, e, pt))
                    while pvq and pvq[0][0] <= t - defer:
                        emit_pv()
                    pvq.extend(new_pvq)
                    if pending:
                        flush_pending()
                while pvq:
                    emit_pv()
                for e in range(2):
                    pending.append(norm_closure(n, j, e, acc[e]))
            while filler:
                filler.popleft()()
        flush_pending()
        for c in oproj_closures(NQ - 1):
            c()


def _build():
    import concourse.mybir as mybir
    import concourse.tile as tile
    from concourse import bacc

    f32 = mybir.dt.float32
    f32r = mybir.dt.float32r
    f8 = mybir.dt.float8e4
    nc = bacc.Bacc("TRN2", target_bir_lowering=False, debug=False,
                   num_devices=NCORES)
    tens = {
        "xT": nc.dram_tensor("xT", [HIDDEN, S], f8, kind="ExternalInput"),
        "xT32": nc.dram_tensor("xT32", [HIDDEN, 512], f32r,
                               kind="ExternalInput"),
        "wqT": nc.dram_tensor("wqT", [HIDDEN, DG], f8, kind="ExternalInput"),
        "wkT": nc.dram_tensor("wkT", [HIDDEN, DG], f8, kind="ExternalInput"),
        "wvT": nc.dram_tensor("wvT", [HIDDEN, DG], f8, kind="ExternalInput"),
        "wqT32": nc.dram_tensor("wqT32", [HIDDEN, DG], f32r,
                                kind="ExternalInput"),
        "wkT32": nc.dram_tensor("wkT32", [HIDDEN, DG], f32r,
                                kind="ExternalInput"),
        "wvT32": nc.dram_tensor("wvT32", [HIDDEN, DG], f32r,
                                kind="ExternalInput"),
        "woT": nc.dram_tensor("woT", [DG, HIDDEN], mybir.dt.bfloat16,
                              kind="ExternalInput"),
        "masks": nc.dram_tensor("masks", [2, P, 1024], mybir.dt.bfloat16,
                                kind="ExternalInput"),
        "out": nc.dram_tensor("out", [S, HIDDEN], f32, kind="ExternalOutput"),
    }
    with tile.TileContext(nc) as tc:
        _emit(nc, tc, tens)
    nc.compile()
    return nc


def get_program():
    if "nc" not in _CACHE:
        _CACHE["nc"] = _build()
    return _CACHE["nc"]


def make_in_maps(hidden_states, attention_mask, wq, wk, wv, wo):
    """Build the per-core input maps (host-side sharding)."""
    import ml_dtypes
    f8 = ml_dtypes.float8_e4m3
    bf = ml_dtypes.bfloat16

    hidden_states = np.asarray(hidden_states, dtype=np.float32)
    attention_mask = np.asarray(attention_mask, dtype=np.float32)
    wq = np.asarray(wq, dtype=np.float32)
    wk = np.asarray(wk, dtype=np.float32)
    wv = np.asarray(wv, dtype=np.float32)
    wo = np.asarray(wo, dtype=np.float32)

    # Pair-level mask tiles for the diagonal blocks of scores^T, derived from
    # the provided additive mask (0 = attend, big negative = blocked).
    am = attention_mask[0, 0]
    mask_np = np.empty((2, P, 1024), dtype=np.float32)
    for t in range(2):
        for u in range(2):
            off = (2 * t + u) * P
            blk = (am[512:1024, 512 + off:512 + off + P] == 0.0)
            mask_np[t, :, u * 512:(u + 1) * 512] = blk.T.astype(np.float32)
    mask_np = mask_np.astype(bf)

    in_maps = []
    for c in range(NCORES):
        b, g = divmod(c, HG)
        rows = slice(g * DG, (g + 1) * DG)
        xt = np.ascontiguousarray(hidden_states[b].T)
        in_maps.append({
            "xT": xt.astype(f8),
            "xT32": np.ascontiguousarray(xt[:, :512]),
            "wqT": np.ascontiguousarray(wq[rows, :].T).astype(f8),
            "wkT": np.ascontiguousarray(wk[rows, :].T).astype(f8),
            "wvT": np.ascontiguousarray(wv[rows, :].T).astype(f8),
            "wqT32": np.ascontiguousarray(wq[rows, :].T),
            "wkT32": np.ascontiguousarray(wk[rows, :].T),
            "wvT32": np.ascontiguousarray(wv[rows, :].T),
            "woT": np.ascontiguousarray(wo[:, rows].T).astype(bf),
            "masks": mask_np,
        })
    return in_maps


def combine_outputs(results):
    out = np.empty((B, S, HIDDEN), dtype=np.float32)
    for b in range(B):
        out[b] = results[HG * b]["out"] + results[HG * b + 1]["out"]
    return out


def kernel(hidden_states, attention_mask, wq, wk, wv, wo):
    from concourse.bass_utils import run_bass_kernel_spmd

    nc = get_program()
    in_maps = make_in_maps(hidden_states, attention_mask, wq, wk, wv, wo)
    res = run_bass_kernel_spmd(nc, in_maps, list(range(NCORES)))
    return combine_outputs(res.results)
